# revision 43
# baseline (speedup 1.0000x reference)
"""CBOW negative-sampling loss kernel for Trainium2 (8 NeuronCores).

Problem (see reference):
    context_embeds = in_W[context].mean(axis=1)          # [B, D]
    true_embeds    = out_W[center.squeeze(1)]            # [B, D]
    pos_loss = softplus(-sum(context_embeds*true_embeds, -1)).mean()
    neg_embeds = out_W[neg_context]                      # [B, K, D]
    neg_loss = softplus(einsum('bkd,bd->bk', ...)).sum(-1).mean()
    out = pos_loss + neg_loss                            # scalar

Strategy: data-parallel over batch across 8 cores (2048 rows each).
Both tables are concatenated into one [200000,128] fp8(e4m3) DRAM
tensor (rows pre-scaled into e4m3's normal range on the host -- the raw
values are all denormal at e4m3); SWDGE indirect DMA gathers rows and
casts them to bf16 on the fly, which cuts the per-descriptor SDMA cost
from ~11 to ~9.7 cycles (the drain is destination-write-bound).  DVE
does the ctx 8->1 add-tree, the w*cs broadcast multiply and the d-fold
tree (all bf16 2x_1P); ACT does exp/ln softplus and the final partial
reductions.  The kernel is DVE-bound: ~40us of vector work against a
~28us gather drain.

Row layout per core: batch row b = chunk*128 + p lives on partition p,
chunk index c in the free dim (16 chunks of 128 rows).  Chunks are
grouped into super-chunks (small first) so compute starts as early as
possible; idx loads are issued as raw pre-TileContext instructions.

The SWDGE gather VERY rarely corrupts a single row (address/timing
dependent, seen on every variant incl. the bf16 baseline; it follows
the DATA not the core).  A max/min clamp on the dots (DVE min/max
suppress NaN) turns that into a ~4e-5 relative error instead of a NaN
-- the old core-rotation retry loop remains only as a backstop.

The walrus build in this container encodes at most ONE semaphore wait
per instruction ("Too many sync wait commands"), rejects the raw-ISA
InstTensorTensorReduce and InstIncSwdgeSem ("ISA wrong length"), and
the runtime crashes on cce_op=add indirect DMA (tried: CCE-accumulated
ctx sums -- dead end).  So: waits are split onto single-wait NoOps at
Tile lowering time (PatchedTileContext below), manual sem updates/waits
are stapled onto instructions post-lowering, and dots use
tensor_tensor + tensor_reduce.
"""

import numpy as np

VOCAB = 100000
DIM = 128
BATCH = 16384
CTX = 8
K_NEG = 10
N_CORES = 8
P = 128

B_CORE = BATCH // N_CORES          # 2048
N_CHUNKS = B_CORE // P             # 16
SUPERS = (1, 3, 6, 6)              # chunks per super-chunk (small first for early start)
W_COLS = 1 + K_NEG                 # center + negatives share the out_W gather

S_IN = 256.0                       # host scale on in_W rows before fp8 quant
S_OUT = 64.0                       # host scale on out_W rows before fp8 quant
DOT_SCALE = CTX * S_IN * S_OUT     # raw dots = DOT_SCALE * true logits

# (dead end, kept for reference): fold the 8-row context sum into the
# gather via CCE-accumulate passes.  Compiles after much effort but the
# runtime crashes on cce_op=add indirect DMA -- do not enable.
CTX_DMA_SUM = False

_CACHE = {}


def _patched_tile_context(lean_exit=True):
    import concourse.mybir as mybir
    import concourse.tile as tile
    from concourse.vector_clock import ScopedClock

    class PatchedTileContext(tile.TileContext):
        """Split multi-wait sync_infos: this container's walrus codegen
        accepts only one semaphore wait (and update) per instruction."""

        def _add_instruction(self, inst):
            si = getattr(inst, "sync_info", None)
            if si is not None and len(si.on_wait) > 1:
                waits = list(si.on_wait)
                for w in waits[:-1]:
                    nop = mybir.InstNoOp(
                        name=f"I-{self.nc.next_id()}-waitsplit",
                        engine=inst.engine,
                        sync_info=mybir.SyncInfo(on_wait=[w], on_update=[]),
                        bass_nofuse=True,
                    )
                    super()._add_instruction(nop)
                inst.sync_info = mybir.SyncInfo(
                    on_wait=[waits[-1]], on_update=list(si.on_update)
                )
            super()._add_instruction(inst)

        def _drain_and_barrier(self, tick_clock, wait_clock):
            drain_inst = self.nc.sync.drain()
            wait_clock.add_sem_waits(
                drain_inst.ins, ScopedClock({None: tick_clock.global_clock})
            )
            si = drain_inst.ins.sync_info
            if si is not None and len(si.on_wait) > 1:
                waits = list(si.on_wait)
                ups = list(si.on_update)
                drain_inst.ins.sync_info = mybir.SyncInfo(
                    on_wait=waits[:1], on_update=[]
                )
                for i, w in enumerate(waits[1:]):
                    d2 = self.nc.sync.drain()
                    last = i == len(waits) - 2
                    d2.ins.sync_info = mybir.SyncInfo(
                        on_wait=[w], on_update=ups if last else []
                    )
            self.nc.all_engine_barrier()
            popped = self.nc._tile_sem_poison_stack.pop()
            assert popped is self._sem_poison
            self.nc.clear_and_free_semaphores(list(self.sems.allocated().values()))
            if not lean_exit:
                self.nc.all_engine_barrier()

    return PatchedTileContext


def build_bass(vocab=VOCAB, supers=SUPERS, ctx_dma_sum=CTX_DMA_SUM, fp8=True):
    """Build the per-core Bass program."""
    import concourse.bass as bass
    import concourse.mybir as mybir

    f32 = mybir.dt.float32
    bf16 = mybir.dt.bfloat16
    i32 = mybir.dt.int32
    tdt = mybir.dt.float8e4 if fp8 else bf16
    n_chunks = sum(supers)
    n_sc = len(supers)
    TileContext = _patched_tile_context()

    nc = bass.Bass()

    c_first = supers[0]
    n_rest = n_chunks - c_first
    idx0_d = nc.dram_tensor("idx0", [P, c_first * (CTX + W_COLS)], i32, kind="ExternalInput")
    idxr_d = nc.dram_tensor("idxr", [P, n_rest * (CTX + W_COLS)], i32, kind="ExternalInput")
    tab_d = nc.dram_tensor("tab", [2 * vocab, DIM], tdt, kind="ExternalInput")
    loss_d = nc.dram_tensor("loss", [P, 2], f32, kind="ExternalOutput")

    # indices before TileContext entry: the idx loads don't depend on the
    # tc-entry protocol (~1.5us of sem/register setup), so issuing them as
    # raw instructions with a pinned sem starts the pipeline that much
    # earlier; consumers get stapled waits after tc exits
    idx0 = nc.alloc_sbuf_tensor("idx0_raw", [P, c_first * (CTX + W_COLS)], i32)
    idxr = nc.alloc_sbuf_tensor("idxr_raw", [P, n_rest * (CTX + W_COLS)], i32)
    idx_sem = nc.alloc_semaphore("idx_sem", num=238)

    def _staple_update(ins, sem_id, sem_name):
        upd = mybir.SyncUpdate(
            sync_type="semaphore", id=sem_id, ant_name=sem_name,
            update_mode="sem-add-imm", update_value=16, update_reg=None,
        )
        si = ins.sync_info
        ins.sync_info = mybir.SyncInfo(
            on_wait=list(si.on_wait) if si else [],
            on_update=(list(si.on_update) if si else []) + [upd],
        )

    i0 = nc.sync.dma_start(out=idx0[:], in_=idx0_d[:])
    _staple_update(i0.ins, 238, "idx_sem")
    ir = nc.sync.dma_start(out=idxr[:], in_=idxr_d[:])
    _staple_update(ir.ins, 238, "idx_sem")

    def ctx_cols(lo, hi):
        if hi <= c_first:
            return idx0[:, lo * CTX:hi * CTX]
        return idxr[:, (lo - c_first) * CTX:(hi - c_first) * CTX]

    def w_cols(lo, hi):
        if hi <= c_first:
            return idx0[:, c_first * CTX + lo * W_COLS:
                        c_first * CTX + hi * W_COLS]
        return idxr[:, n_rest * CTX + (lo - c_first) * W_COLS:
                    n_rest * CTX + (hi - c_first) * W_COLS]

    with TileContext(nc) as tc:
        with (
            nc.allow_low_precision(reason="quantized dots are well within tolerance"),
            tc.tile_pool(name="gather", bufs=1) as gpool,
            tc.tile_pool(name="work", bufs=3) as wpool,
            tc.tile_pool(name="accp", bufs=1) as apool,
        ):

            acc = apool.tile([P, n_sc], f32)                    # softplus sums
            dots_all = apool.tile([P, n_chunks * W_COLS], f32)  # raw dots

            ctx_tiles = []
            if ctx_dma_sum:
                cs2 = nc.alloc_sbuf_tensor("cs2_raw", [P, n_chunks * 2 * DIM], bf16)
                cs1 = nc.alloc_sbuf_tensor("cs1_raw", [P, n_chunks * DIM], bf16)
                # One sem per super (the Tile scheduler interleaves the DMA
                # stream across supers, so a single cumulative count would
                # fire early).  Pinned high so the tile pool's lazily
                # allocated sems stay one contiguous range (the exit
                # RANGE_CLEAR encodes a single range).
                ctx_sems = [
                    nc.alloc_semaphore(f"ctx_sum_{s}", num=240 + s)
                    for s in range(n_sc)
                ]
            ctx_pass_fixups = []
            ctx_wait_fixups = []
            ctx_add_names = []

            # issue ALL gathers first so the Pool engine streams descriptors
            # back-to-back and the SDMA queue never starves; ctx BEFORE w in
            # each super so the DVE ctx tree starts while w is still draining
            w_tiles = []
            c0 = 0
            for s, sc in enumerate(supers):
                need = 16 if c0 + sc <= c_first else 32
                if not ctx_dma_sum:
                    ctx_g = gpool.tile([P, sc * CTX * DIM], bf16, tag=f"ctx_g{s}")
                    gi = nc.gpsimd.indirect_dma_start(
                        out=ctx_g[:],
                        out_offset=None,
                        in_=tab_d[:],
                        in_offset=bass.IndirectOffsetOnAxis(
                            ap=ctx_cols(c0, c0 + sc), axis=0
                        ),
                    )
                    ctx_wait_fixups.append((gi.ins, 238, "idx_sem", need))
                    ctx_tiles.append(ctx_g)
                w_g = gpool.tile([P, sc * W_COLS * DIM], bf16, tag=f"w_g{s}")
                wi = nc.gpsimd.indirect_dma_start(
                    out=w_g[:],
                    out_offset=None,
                    in_=tab_d[:],
                    in_offset=bass.IndirectOffsetOnAxis(
                        ap=w_cols(c0, c0 + sc), axis=0
                    ),
                )
                ctx_wait_fixups.append((wi.ins, 238, "idx_sem", need))
                w_tiles.append(w_g)
                if ctx_dma_sum:
                    kv = ctx_cols(c0, c0 + sc).rearrange(
                        "p (c k) -> p c k", k=CTX
                    )
                    prev = None
                    for j in range(CTX // 2):
                        inst = nc.gpsimd.indirect_dma_start(
                            out=cs2[:, c0 * 2 * DIM:(c0 + sc) * 2 * DIM],
                            out_offset=None,
                            in_=tab_d[:],
                            in_offset=bass.IndirectOffsetOnAxis(
                                ap=kv[:, :, 2 * j:2 * j + 2], axis=0
                            ),
                            compute_op=(
                                mybir.AluOpType.bypass if j == 0
                                else mybir.AluOpType.add
                            ),
                        )
                        # completion update is attached AFTER TileContext
                        # exit (see ctx_pass_fixups below): updates staged
                        # before Tile's lowering get converted into
                        # InstIncSwdgeSem companions this walrus can't encode
                        ctx_pass_fixups.append((inst.ins, 240 + s, f"ctx_sum_{s}"))
                        # CCE accumulate requires pass j to land after pass
                        # j-1; the queue is FIFO, so pin the EMIT order with
                        # scheduler-only (no-semaphore) dependency edges
                        if prev is not None:
                            deps = bass.InstructionNameOrderedSet()
                            deps.add(prev)
                            inst.ins.add_nosync_dependencies_from(deps)
                        prev = inst.ins.name
                c0 += sc

            c0 = 0
            for s, sc in enumerate(supers):
                w_g = w_tiles[s]
                if ctx_dma_sum:
                    c2v = cs2[:, c0 * 2 * DIM:(c0 + sc) * 2 * DIM].rearrange(
                        "p (c o d) -> p c o d", c=sc, o=2
                    )
                    cs_s = cs1[:, c0 * DIM:(c0 + sc) * DIM]
                    add_inst = nc.vector.tensor_add(
                        out=cs_s.rearrange("p (c o d) -> p c o d", c=sc, o=1),
                        in0=c2v[:, :, 0:1, :], in1=c2v[:, :, 1:2, :],
                    )
                    # runtime gate (stapled post-exit so the schedule sim,
                    # which can't see the matching updates, won't deadlock)
                    ctx_wait_fixups.append(
                        (add_inst.ins, 240 + s, f"ctx_sum_{s}", 16 * (CTX // 2))
                    )
                    ctx_add_names.append(add_inst.ins.name)
                else:
                    ctx_g = ctx_tiles[s]
                    cv = ctx_g[:].rearrange("p (c k d) -> p c k d", c=sc, k=CTX)
                    t1 = wpool.tile([P, sc * 4 * DIM], bf16, tag="t1")
                    t1v = t1[:].rearrange("p (c k d) -> p c k d", c=sc, k=4)
                    nc.vector.tensor_add(out=t1v, in0=cv[:, :, 0:4, :], in1=cv[:, :, 4:8, :])
                    t2 = wpool.tile([P, sc * 2 * DIM], bf16, tag="t2")
                    t2v = t2[:].rearrange("p (c k d) -> p c k d", c=sc, k=2)
                    nc.vector.tensor_add(out=t2v, in0=t1v[:, :, 0:2, :], in1=t1v[:, :, 2:4, :])
                    cs = wpool.tile([P, sc * DIM], bf16, tag="cs")
                    csv = cs[:].rearrange("p (c o d) -> p c o d", c=sc, o=1)
                    nc.vector.tensor_add(out=csv, in0=t2v[:, :, 0:1, :], in1=t2v[:, :, 1:2, :])
                    cs_s = cs[:]

                # prod[p, c, t, d] = w_g[p, c, t, d] * cs[p, c, d]
                prod = wpool.tile([P, sc * W_COLS * DIM], bf16, tag="prod")
                mul_inst = nc.vector.tensor_mul(
                    out=prod[:],
                    in0=w_g[:],
                    in1=cs_s.rearrange("p (c o d) -> p c o d", c=sc, o=1).broadcast_to(
                        [P, sc, W_COLS, DIM]
                    ),
                )
                if ctx_dma_sum:
                    # cs1 is untracked: pin mul after the ctx-add by a
                    # scheduler-only edge (runtime safety comes from the add's
                    # stapled sem wait + same-engine program order)
                    deps = bass.InstructionNameOrderedSet()
                    deps.add(ctx_add_names[-1])
                    mul_inst.ins.add_nosync_dependencies_from(deps)
                # fold d 128 -> 16 with adds (2x mode) before the 1x reduce
                pv = prod[:].rearrange("p (c t h d) -> p c t h d", c=sc, t=W_COLS, h=2)
                f1 = wpool.tile([P, sc * W_COLS * 64], bf16, tag="f1")
                f1v = f1[:].rearrange("p (c t h d) -> p c t h d", c=sc, t=W_COLS, h=2)
                nc.vector.tensor_add(
                    out=f1[:].rearrange("p (c t d) -> p c t d", c=sc, t=W_COLS),
                    in0=pv[:, :, :, 0, :], in1=pv[:, :, :, 1, :],
                )
                f2 = wpool.tile([P, sc * W_COLS * 32], bf16, tag="f2")
                nc.vector.tensor_add(
                    out=f2[:].rearrange("p (c t d) -> p c t d", c=sc, t=W_COLS),
                    in0=f1v[:, :, :, 0, :], in1=f1v[:, :, :, 1, :],
                )
                f2v = f2[:].rearrange("p (c t h d) -> p c t h d", c=sc, t=W_COLS, h=2)
                f3 = wpool.tile([P, sc * W_COLS * 16], bf16, tag="f3")
                nc.vector.tensor_add(
                    out=f3[:].rearrange("p (c t d) -> p c t d", c=sc, t=W_COLS),
                    in0=f2v[:, :, :, 0, :], in1=f2v[:, :, :, 1, :],
                )
                dots = dots_all[:, c0 * W_COLS:(c0 + sc) * W_COLS]
                nc.vector.reduce_sum(
                    out=dots,
                    in_=f3[:].rearrange("p (c t d) -> p c t d", c=sc, t=W_COLS),
                    axis=mybir.AxisListType.X,
                )

                # The SWDGE gather VERY rarely corrupts a single row (NaN
                # garbage, address-pattern/timing dependent -- seen on every
                # kernel variant incl. the bf16 baseline).  One bad row is
                # numerically irrelevant at 2e-2 tolerance (~4e-5 rel), but a
                # NaN would poison the whole accumulation: clamp the dots so
                # NaN -> +/-5e4 -> softplus contributes a tiny finite term.
                nc.vector.tensor_scalar(
                    out=dots, in0=dots,
                    scalar1=-5.0e4, scalar2=5.0e4,
                    op0=mybir.AluOpType.max, op1=mybir.AluOpType.min,
                )
                # softplus identity: softplus(-x) = softplus(x) - x, so apply
                # softplus(dots/DOT_SCALE) to ALL 11 columns (contiguous ACT
                # ops) and subtract the pos dots at the end (host combines).
                es = wpool.tile([P, sc * W_COLS], f32, tag="es")
                sp = wpool.tile([P, sc * W_COLS], f32, tag="sp")
                nc.scalar.activation(
                    out=es[:], in_=dots,
                    func=mybir.ActivationFunctionType.Exp, scale=1.0 / DOT_SCALE,
                )
                nc.scalar.activation(
                    out=sp[:], in_=es[:],
                    func=mybir.ActivationFunctionType.Ln, bias=1.0,
                    accum_out=acc[:, s:s + 1],
                )
                c0 += sc

            # partials [p, 0] = sum of softplus terms, [p, 1] = sum of raw
            # pos dots; host: (sum0 - sum1/DOT_SCALE) / BATCH.  Both final
            # reductions run on ACT (activation Copy + accum_out) so the DVE
            # tail ends at the last fold and the output DMA fires from the
            # otherwise-idle Scalar queue's results.
            partials = apool.tile([P, 2], f32)
            scr_a = wpool.tile([P, n_sc], f32, tag="scr_a")
            nc.scalar.activation(
                out=scr_a[:], in_=acc[:],
                func=mybir.ActivationFunctionType.Copy,
                accum_out=partials[:, 0:1],
            )
            scr_p = wpool.tile([P, n_chunks], f32, tag="scr_p")
            nc.scalar.activation(
                out=scr_p[:].rearrange("p (o c) -> p o c", o=1),
                in_=dots_all[:].rearrange("p (c t) -> p t c", t=W_COLS)[:, 0:1, :],
                func=mybir.ActivationFunctionType.Copy,
                accum_out=partials[:, 1:2],
            )
            nc.sync.dma_start(out=loss_d[:], in_=partials[:])

    for ins, sem_id, sem_name in ctx_pass_fixups:
        upd = mybir.SyncUpdate(
            sync_type="semaphore", id=sem_id, ant_name=sem_name,
            update_mode="sem-add-imm", update_value=16, update_reg=None,
        )
        si = ins.sync_info
        if si is None:
            ins.sync_info = mybir.SyncInfo(on_wait=[], on_update=[upd])
        else:
            ins.sync_info = mybir.SyncInfo(
                on_wait=list(si.on_wait),
                on_update=list(si.on_update) + [upd],
            )

    if ctx_wait_fixups:
        blocks_of = {}
        for b in nc.m.functions[0].blocks:
            for i in b.instructions:
                blocks_of[i.name] = b
        for ins, sem_id, sem_name, val in ctx_wait_fixups:
            w = mybir.SyncWait(
                sync_type="semaphore", id=sem_id, ant_name=sem_name,
                wait_mode="sem-ge-imm", wait_value=val,
            )
            si = ins.sync_info
            if si is None or not si.on_wait:
                ins.sync_info = mybir.SyncInfo(
                    on_wait=[w],
                    on_update=list(si.on_update) if si else [],
                )
            else:
                # single-wait walrus: put our wait on a NoOp just before
                b = blocks_of[ins.name]
                pos = b.instructions.index(ins)
                nop = mybir.InstNoOp(
                    name=f"I-{nc.next_id()}-ctxwait",
                    engine=ins.engine,
                    sync_info=mybir.SyncInfo(on_wait=[w], on_update=[]),
                    bass_nofuse=True,
                )
                b.instructions.insert(pos, nop)

    nc.finalize()
    return nc


def pack_indices(center, context, neg_context, n_chunks=N_CHUNKS):
    """Pack per-core indices into the SBUF layouts the kernel expects.

    ctx_idx [P, n_chunks*CTX]: [p, c*CTX + k] = context[c*128 + p, k]
    w_idx   [P, n_chunks*11]:  [p, c*11 + 0] = center row, +1.. = negatives
    (w indices are offset by VOCAB into the concatenated table)
    """
    rows = n_chunks * P
    ctx_l, w_l = [], []
    for m in range(N_CORES):
        lo = m * rows
        ctx = np.ascontiguousarray(context[lo:lo + rows]).astype(np.int32)
        cen = np.ascontiguousarray(center[lo:lo + rows]).astype(np.int32)
        neg = np.ascontiguousarray(neg_context[lo:lo + rows]).astype(np.int32)
        ctx_p = ctx.reshape(n_chunks, P, CTX).transpose(1, 0, 2).reshape(P, n_chunks * CTX)
        w = np.concatenate([cen.reshape(rows, 1), neg.reshape(rows, K_NEG)], axis=1) + VOCAB
        w_p = w.reshape(n_chunks, P, W_COLS).transpose(1, 0, 2).reshape(P, n_chunks * W_COLS)
        ctx_l.append(np.ascontiguousarray(ctx_p))
        w_l.append(np.ascontiguousarray(w_p.astype(np.int32)))
    return ctx_l, w_l


def make_table(in_W, out_W):
    """fp8 e4m3 concat table; rows pre-scaled into e4m3's normal range
    (the raw values are denormal at fp8 and would lose most precision)."""
    import ml_dtypes

    tab = np.empty((2 * VOCAB, DIM), dtype=ml_dtypes.float8_e4m3)
    tab[:VOCAB] = (np.asarray(in_W, dtype=np.float32) * S_IN).astype(ml_dtypes.float8_e4m3)
    tab[VOCAB:] = (np.asarray(out_W, dtype=np.float32) * S_OUT).astype(ml_dtypes.float8_e4m3)
    return np.ascontiguousarray(tab)


def make_in_maps(center, context, neg_context, in_W, out_W):
    ctx_l, w_l = pack_indices(np.asarray(center), np.asarray(context), np.asarray(neg_context))
    tab = make_table(in_W, out_W)
    cf = SUPERS[0]
    in_maps = []
    for m in range(N_CORES):
        c, w = ctx_l[m], w_l[m]
        idx0 = np.ascontiguousarray(
            np.concatenate([c[:, :cf * CTX], w[:, :cf * W_COLS]], axis=1))
        idxr = np.ascontiguousarray(
            np.concatenate([c[:, cf * CTX:], w[:, cf * W_COLS:]], axis=1))
        in_maps.append({"idx0": idx0, "idxr": idxr, "tab": tab})
    return in_maps


def kernel(center, context, neg_context, in_W, out_W):
    from concourse.bass_utils import run_bass_kernel_spmd

    if "nc" not in _CACHE:
        _CACHE["nc"] = build_bass()
    nc = _CACHE["nc"]

    in_maps = make_in_maps(center, context, neg_context, in_W, out_W)
    # Rare per-core HW corruption (can be sticky on a given core) shows up
    # as NaN partials.  Retry with the slice->core assignment ROTATED each
    # attempt so a slice pinned to a bad core is recomputed by a good one.
    vals = np.full(N_CORES, np.nan)
    for attempt in range(2 * N_CORES):
        rot = attempt % N_CORES
        maps = [None] * N_CORES
        for s in range(N_CORES):
            maps[(s + rot) % N_CORES] = in_maps[s]
        res = run_bass_kernel_spmd(nc, maps, core_ids=list(range(N_CORES)))
        for s in range(N_CORES):
            if not np.isfinite(vals[s]):
                part = np.asarray(
                    res.results[(s + rot) % N_CORES]["loss"], dtype=np.float64
                )
                v = part[:, 0].sum() - part[:, 1].sum() / DOT_SCALE
                if np.isfinite(v):
                    vals[s] = v
        if np.isfinite(vals).all():
            break
    return np.float32(vals.sum() / BATCH)


# revision 46
# speedup vs baseline: 1.0435x; 1.0435x over previous
"""CBOW negative-sampling loss kernel for Trainium2 (8 NeuronCores).

Problem (see reference):
    context_embeds = in_W[context].mean(axis=1)          # [B, D]
    true_embeds    = out_W[center.squeeze(1)]            # [B, D]
    pos_loss = softplus(-sum(context_embeds*true_embeds, -1)).mean()
    neg_embeds = out_W[neg_context]                      # [B, K, D]
    neg_loss = softplus(einsum('bkd,bd->bk', ...)).sum(-1).mean()
    out = pos_loss + neg_loss                            # scalar

Strategy: data-parallel over batch across 8 cores (2048 rows each).
Both tables are concatenated into one [200000,128] fp8(e4m3) DRAM
tensor (rows pre-scaled into e4m3's normal range on the host -- the raw
values are all denormal at e4m3); SWDGE indirect DMA gathers rows and
casts them to bf16 on the fly, which cuts the per-descriptor SDMA cost
from ~11 to ~9.7 cycles (the drain is destination-write-bound).  DVE
does the ctx 8->1 add-tree, the w*cs broadcast multiply and the d-fold
tree (all bf16 2x_1P); ACT does exp/ln softplus and the final partial
reductions.  The kernel is DVE-bound: ~40us of vector work against a
~28us gather drain.

Row layout per core: batch row b = chunk*128 + p lives on partition p,
chunk index c in the free dim (16 chunks of 128 rows).  Chunks are
grouped into super-chunks (small first) so compute starts as early as
possible; idx loads are issued as raw pre-TileContext instructions.

The SWDGE gather VERY rarely corrupts a single row (address/timing
dependent, seen on every variant incl. the bf16 baseline; it follows
the DATA not the core).  A max/min clamp on the dots (DVE min/max
suppress NaN) turns that into a ~4e-5 relative error instead of a NaN
-- the old core-rotation retry loop remains only as a backstop.

The walrus build in this container encodes at most ONE semaphore wait
per instruction ("Too many sync wait commands"), rejects the raw-ISA
InstTensorTensorReduce and InstIncSwdgeSem ("ISA wrong length"), and
the runtime crashes on cce_op=add indirect DMA (tried: CCE-accumulated
ctx sums -- dead end).  So: waits are split onto single-wait NoOps at
Tile lowering time (PatchedTileContext below), manual sem updates/waits
are stapled onto instructions post-lowering, and dots use
tensor_tensor + tensor_reduce.
"""

import numpy as np

VOCAB = 100000
DIM = 128
BATCH = 16384
CTX = 8
K_NEG = 10
N_CORES = 8
P = 128

B_CORE = BATCH // N_CORES          # 2048
N_CHUNKS = B_CORE // P             # 16
SUPERS = (1, 3, 6, 6)              # chunks per super-chunk (small first for early start)
W_COLS = 1 + K_NEG                 # center + negatives share the out_W gather

S_IN = 256.0                       # host scale on in_W rows before fp8 quant
S_OUT = 64.0                       # host scale on out_W rows before fp8 quant
DOT_SCALE = CTX * S_IN * S_OUT     # raw dots = DOT_SCALE * true logits

# (dead end, kept for reference): fold the 8-row context sum into the
# gather via CCE-accumulate passes.  Compiles after much effort but the
# runtime crashes on cce_op=add indirect DMA -- do not enable.
CTX_DMA_SUM = False

_CACHE = {}


def _patched_tile_context(lean_exit=True):
    import concourse.mybir as mybir
    import concourse.tile as tile
    from concourse.vector_clock import ScopedClock

    class PatchedTileContext(tile.TileContext):
        """Split multi-wait sync_infos: this container's walrus codegen
        accepts only one semaphore wait (and update) per instruction."""

        def _add_instruction(self, inst):
            si = getattr(inst, "sync_info", None)
            if si is not None and len(si.on_wait) > 1:
                waits = list(si.on_wait)
                for w in waits[:-1]:
                    nop = mybir.InstNoOp(
                        name=f"I-{self.nc.next_id()}-waitsplit",
                        engine=inst.engine,
                        sync_info=mybir.SyncInfo(on_wait=[w], on_update=[]),
                        bass_nofuse=True,
                    )
                    super()._add_instruction(nop)
                inst.sync_info = mybir.SyncInfo(
                    on_wait=[waits[-1]], on_update=list(si.on_update)
                )
            super()._add_instruction(inst)

        def _drain_and_barrier(self, tick_clock, wait_clock):
            drain_inst = self.nc.sync.drain()
            wait_clock.add_sem_waits(
                drain_inst.ins, ScopedClock({None: tick_clock.global_clock})
            )
            si = drain_inst.ins.sync_info
            if si is not None and len(si.on_wait) > 1:
                waits = list(si.on_wait)
                ups = list(si.on_update)
                drain_inst.ins.sync_info = mybir.SyncInfo(
                    on_wait=waits[:1], on_update=[]
                )
                for i, w in enumerate(waits[1:]):
                    d2 = self.nc.sync.drain()
                    last = i == len(waits) - 2
                    d2.ins.sync_info = mybir.SyncInfo(
                        on_wait=[w], on_update=ups if last else []
                    )
            self.nc.all_engine_barrier()
            popped = self.nc._tile_sem_poison_stack.pop()
            assert popped is self._sem_poison
            self.nc.clear_and_free_semaphores(list(self.sems.allocated().values()))
            if not lean_exit:
                self.nc.all_engine_barrier()

    return PatchedTileContext


def build_bass(vocab=VOCAB, supers=SUPERS, ctx_dma_sum=CTX_DMA_SUM, fp8=True):
    """Build the per-core Bass program."""
    import concourse.bass as bass
    import concourse.mybir as mybir

    f32 = mybir.dt.float32
    bf16 = mybir.dt.bfloat16
    i32 = mybir.dt.int32
    tdt = mybir.dt.float8e4 if fp8 else bf16
    n_chunks = sum(supers)
    n_sc = len(supers)
    TileContext = _patched_tile_context()

    nc = bass.Bass()

    c_first = supers[0]
    n_rest = n_chunks - c_first
    idx0_d = nc.dram_tensor("idx0", [P, c_first * (CTX + W_COLS)], i32, kind="ExternalInput")
    idxr_d = nc.dram_tensor("idxr", [P, n_rest * (CTX + W_COLS)], i32, kind="ExternalInput")
    tab_d = nc.dram_tensor("tab", [2 * vocab, DIM], tdt, kind="ExternalInput")
    loss_d = nc.dram_tensor("loss", [P, 2 * len(supers)], f32, kind="ExternalOutput")

    # indices before TileContext entry: the idx loads don't depend on the
    # tc-entry protocol (~1.5us of sem/register setup), so issuing them as
    # raw instructions with a pinned sem starts the pipeline that much
    # earlier; consumers get stapled waits after tc exits
    idx0 = nc.alloc_sbuf_tensor("idx0_raw", [P, c_first * (CTX + W_COLS)], i32)
    idxr = nc.alloc_sbuf_tensor("idxr_raw", [P, n_rest * (CTX + W_COLS)], i32)
    idx_sem = nc.alloc_semaphore("idx_sem", num=238)

    def _staple_update(ins, sem_id, sem_name):
        upd = mybir.SyncUpdate(
            sync_type="semaphore", id=sem_id, ant_name=sem_name,
            update_mode="sem-add-imm", update_value=16, update_reg=None,
        )
        si = ins.sync_info
        ins.sync_info = mybir.SyncInfo(
            on_wait=list(si.on_wait) if si else [],
            on_update=(list(si.on_update) if si else []) + [upd],
        )

    i0 = nc.sync.dma_start(out=idx0[:], in_=idx0_d[:])
    _staple_update(i0.ins, 238, "idx_sem")
    ir = nc.sync.dma_start(out=idxr[:], in_=idxr_d[:])
    _staple_update(ir.ins, 238, "idx_sem")

    def ctx_cols(lo, hi):
        if hi <= c_first:
            return idx0[:, lo * CTX:hi * CTX]
        return idxr[:, (lo - c_first) * CTX:(hi - c_first) * CTX]

    def w_cols(lo, hi):
        if hi <= c_first:
            return idx0[:, c_first * CTX + lo * W_COLS:
                        c_first * CTX + hi * W_COLS]
        return idxr[:, n_rest * CTX + (lo - c_first) * W_COLS:
                    n_rest * CTX + (hi - c_first) * W_COLS]

    with TileContext(nc) as tc:
        with (
            nc.allow_low_precision(reason="quantized dots are well within tolerance"),
            tc.tile_pool(name="gather", bufs=1) as gpool,
            tc.tile_pool(name="work", bufs=3) as wpool,
            tc.tile_pool(name="accp", bufs=1) as apool,
        ):

            # per-super partial outputs: [:, 2s] = softplus sum, [:, 2s+1] =
            # raw pos-dot sum; each super DMAs its pair out as soon as its
            # ACT ops finish, so only the LAST super's accums + one small DMA
            # sit on the tail (host sums the 2*n_sc columns per partition)
            partials = apool.tile([P, 2 * n_sc], f32)
            dots_all = apool.tile([P, n_chunks * W_COLS], f32)  # raw dots

            ctx_tiles = []
            if ctx_dma_sum:
                cs2 = nc.alloc_sbuf_tensor("cs2_raw", [P, n_chunks * 2 * DIM], bf16)
                cs1 = nc.alloc_sbuf_tensor("cs1_raw", [P, n_chunks * DIM], bf16)
                # One sem per super (the Tile scheduler interleaves the DMA
                # stream across supers, so a single cumulative count would
                # fire early).  Pinned high so the tile pool's lazily
                # allocated sems stay one contiguous range (the exit
                # RANGE_CLEAR encodes a single range).
                ctx_sems = [
                    nc.alloc_semaphore(f"ctx_sum_{s}", num=240 + s)
                    for s in range(n_sc)
                ]
            ctx_pass_fixups = []
            ctx_wait_fixups = []
            ctx_add_names = []

            # issue ALL gathers first so the Pool engine streams descriptors
            # back-to-back and the SDMA queue never starves; ctx BEFORE w in
            # each super so the DVE ctx tree starts while w is still draining
            w_tiles = []
            c0 = 0
            for s, sc in enumerate(supers):
                need = 16 if c0 + sc <= c_first else 32
                if not ctx_dma_sum:
                    ctx_g = gpool.tile([P, sc * CTX * DIM], bf16, tag=f"ctx_g{s}")
                    gi = nc.gpsimd.indirect_dma_start(
                        out=ctx_g[:],
                        out_offset=None,
                        in_=tab_d[:],
                        in_offset=bass.IndirectOffsetOnAxis(
                            ap=ctx_cols(c0, c0 + sc), axis=0
                        ),
                    )
                    ctx_wait_fixups.append((gi.ins, 238, "idx_sem", need))
                    ctx_tiles.append(ctx_g)
                w_g = gpool.tile([P, sc * W_COLS * DIM], bf16, tag=f"w_g{s}")
                wi = nc.gpsimd.indirect_dma_start(
                    out=w_g[:],
                    out_offset=None,
                    in_=tab_d[:],
                    in_offset=bass.IndirectOffsetOnAxis(
                        ap=w_cols(c0, c0 + sc), axis=0
                    ),
                )
                ctx_wait_fixups.append((wi.ins, 238, "idx_sem", need))
                w_tiles.append(w_g)
                if ctx_dma_sum:
                    kv = ctx_cols(c0, c0 + sc).rearrange(
                        "p (c k) -> p c k", k=CTX
                    )
                    prev = None
                    for j in range(CTX // 2):
                        inst = nc.gpsimd.indirect_dma_start(
                            out=cs2[:, c0 * 2 * DIM:(c0 + sc) * 2 * DIM],
                            out_offset=None,
                            in_=tab_d[:],
                            in_offset=bass.IndirectOffsetOnAxis(
                                ap=kv[:, :, 2 * j:2 * j + 2], axis=0
                            ),
                            compute_op=(
                                mybir.AluOpType.bypass if j == 0
                                else mybir.AluOpType.add
                            ),
                        )
                        # completion update is attached AFTER TileContext
                        # exit (see ctx_pass_fixups below): updates staged
                        # before Tile's lowering get converted into
                        # InstIncSwdgeSem companions this walrus can't encode
                        ctx_pass_fixups.append((inst.ins, 240 + s, f"ctx_sum_{s}"))
                        # CCE accumulate requires pass j to land after pass
                        # j-1; the queue is FIFO, so pin the EMIT order with
                        # scheduler-only (no-semaphore) dependency edges
                        if prev is not None:
                            deps = bass.InstructionNameOrderedSet()
                            deps.add(prev)
                            inst.ins.add_nosync_dependencies_from(deps)
                        prev = inst.ins.name
                c0 += sc

            c0 = 0
            for s, sc in enumerate(supers):
                w_g = w_tiles[s]
                if ctx_dma_sum:
                    c2v = cs2[:, c0 * 2 * DIM:(c0 + sc) * 2 * DIM].rearrange(
                        "p (c o d) -> p c o d", c=sc, o=2
                    )
                    cs_s = cs1[:, c0 * DIM:(c0 + sc) * DIM]
                    add_inst = nc.vector.tensor_add(
                        out=cs_s.rearrange("p (c o d) -> p c o d", c=sc, o=1),
                        in0=c2v[:, :, 0:1, :], in1=c2v[:, :, 1:2, :],
                    )
                    # runtime gate (stapled post-exit so the schedule sim,
                    # which can't see the matching updates, won't deadlock)
                    ctx_wait_fixups.append(
                        (add_inst.ins, 240 + s, f"ctx_sum_{s}", 16 * (CTX // 2))
                    )
                    ctx_add_names.append(add_inst.ins.name)
                else:
                    ctx_g = ctx_tiles[s]
                    cv = ctx_g[:].rearrange("p (c k d) -> p c k d", c=sc, k=CTX)
                    t1 = wpool.tile([P, sc * 4 * DIM], bf16, tag="t1")
                    t1v = t1[:].rearrange("p (c k d) -> p c k d", c=sc, k=4)
                    nc.vector.tensor_add(out=t1v, in0=cv[:, :, 0:4, :], in1=cv[:, :, 4:8, :])
                    t2 = wpool.tile([P, sc * 2 * DIM], bf16, tag="t2")
                    t2v = t2[:].rearrange("p (c k d) -> p c k d", c=sc, k=2)
                    nc.vector.tensor_add(out=t2v, in0=t1v[:, :, 0:2, :], in1=t1v[:, :, 2:4, :])
                    cs = wpool.tile([P, sc * DIM], bf16, tag="cs")
                    csv = cs[:].rearrange("p (c o d) -> p c o d", c=sc, o=1)
                    nc.vector.tensor_add(out=csv, in0=t2v[:, :, 0:1, :], in1=t2v[:, :, 1:2, :])
                    cs_s = cs[:]

                # prod[p, c, t, d] = w_g[p, c, t, d] * cs[p, c, d]
                prod = wpool.tile([P, sc * W_COLS * DIM], bf16, tag="prod")
                mul_inst = nc.vector.tensor_mul(
                    out=prod[:],
                    in0=w_g[:],
                    in1=cs_s.rearrange("p (c o d) -> p c o d", c=sc, o=1).broadcast_to(
                        [P, sc, W_COLS, DIM]
                    ),
                )
                if ctx_dma_sum:
                    # cs1 is untracked: pin mul after the ctx-add by a
                    # scheduler-only edge (runtime safety comes from the add's
                    # stapled sem wait + same-engine program order)
                    deps = bass.InstructionNameOrderedSet()
                    deps.add(ctx_add_names[-1])
                    mul_inst.ins.add_nosync_dependencies_from(deps)
                # fold d 128 -> 16 with adds (2x mode) before the 1x reduce
                pv = prod[:].rearrange("p (c t h d) -> p c t h d", c=sc, t=W_COLS, h=2)
                f1 = wpool.tile([P, sc * W_COLS * 64], bf16, tag="f1")
                f1v = f1[:].rearrange("p (c t h d) -> p c t h d", c=sc, t=W_COLS, h=2)
                nc.vector.tensor_add(
                    out=f1[:].rearrange("p (c t d) -> p c t d", c=sc, t=W_COLS),
                    in0=pv[:, :, :, 0, :], in1=pv[:, :, :, 1, :],
                )
                f2 = wpool.tile([P, sc * W_COLS * 32], bf16, tag="f2")
                nc.vector.tensor_add(
                    out=f2[:].rearrange("p (c t d) -> p c t d", c=sc, t=W_COLS),
                    in0=f1v[:, :, :, 0, :], in1=f1v[:, :, :, 1, :],
                )
                f2v = f2[:].rearrange("p (c t h d) -> p c t h d", c=sc, t=W_COLS, h=2)
                f3 = wpool.tile([P, sc * W_COLS * 16], bf16, tag="f3")
                nc.vector.tensor_add(
                    out=f3[:].rearrange("p (c t d) -> p c t d", c=sc, t=W_COLS),
                    in0=f2v[:, :, :, 0, :], in1=f2v[:, :, :, 1, :],
                )
                dots = dots_all[:, c0 * W_COLS:(c0 + sc) * W_COLS]
                nc.vector.reduce_sum(
                    out=dots,
                    in_=f3[:].rearrange("p (c t d) -> p c t d", c=sc, t=W_COLS),
                    axis=mybir.AxisListType.X,
                )

                # The SWDGE gather VERY rarely corrupts a single row (NaN
                # garbage, address-pattern/timing dependent -- seen on every
                # kernel variant incl. the bf16 baseline).  One bad row is
                # numerically irrelevant at 2e-2 tolerance (~4e-5 rel), but a
                # NaN would poison the whole accumulation: clamp the dots so
                # NaN -> +/-5e4 -> softplus contributes a tiny finite term.
                nc.vector.tensor_scalar(
                    out=dots, in0=dots,
                    scalar1=-5.0e4, scalar2=5.0e4,
                    op0=mybir.AluOpType.max, op1=mybir.AluOpType.min,
                )
                # softplus identity: softplus(-x) = softplus(x) - x, so apply
                # softplus(dots/DOT_SCALE) to ALL 11 columns (contiguous ACT
                # ops) and subtract the pos dots at the end (host combines).
                es = wpool.tile([P, sc * W_COLS], f32, tag="es")
                sp = wpool.tile([P, sc * W_COLS], f32, tag="sp")
                nc.scalar.activation(
                    out=es[:], in_=dots,
                    func=mybir.ActivationFunctionType.Exp, scale=1.0 / DOT_SCALE,
                )
                nc.scalar.activation(
                    out=sp[:], in_=es[:],
                    func=mybir.ActivationFunctionType.Ln, bias=1.0,
                    accum_out=partials[:, 2 * s:2 * s + 1],
                )
                scr_p = wpool.tile([P, sc], f32, tag="scr_p")
                nc.scalar.activation(
                    out=scr_p[:].rearrange("p (o c) -> p o c", o=1),
                    in_=dots.rearrange("p (c t) -> p t c", t=W_COLS)[:, 0:1, :],
                    func=mybir.ActivationFunctionType.Copy,
                    accum_out=partials[:, 2 * s + 1:2 * s + 2],
                )
                nc.sync.dma_start(
                    out=loss_d[:, 2 * s:2 * s + 2],
                    in_=partials[:, 2 * s:2 * s + 2],
                )
                c0 += sc


    for ins, sem_id, sem_name in ctx_pass_fixups:
        upd = mybir.SyncUpdate(
            sync_type="semaphore", id=sem_id, ant_name=sem_name,
            update_mode="sem-add-imm", update_value=16, update_reg=None,
        )
        si = ins.sync_info
        if si is None:
            ins.sync_info = mybir.SyncInfo(on_wait=[], on_update=[upd])
        else:
            ins.sync_info = mybir.SyncInfo(
                on_wait=list(si.on_wait),
                on_update=list(si.on_update) + [upd],
            )

    if ctx_wait_fixups:
        blocks_of = {}
        for b in nc.m.functions[0].blocks:
            for i in b.instructions:
                blocks_of[i.name] = b
        for ins, sem_id, sem_name, val in ctx_wait_fixups:
            w = mybir.SyncWait(
                sync_type="semaphore", id=sem_id, ant_name=sem_name,
                wait_mode="sem-ge-imm", wait_value=val,
            )
            si = ins.sync_info
            if si is None or not si.on_wait:
                ins.sync_info = mybir.SyncInfo(
                    on_wait=[w],
                    on_update=list(si.on_update) if si else [],
                )
            else:
                # single-wait walrus: put our wait on a NoOp just before
                b = blocks_of[ins.name]
                pos = b.instructions.index(ins)
                nop = mybir.InstNoOp(
                    name=f"I-{nc.next_id()}-ctxwait",
                    engine=ins.engine,
                    sync_info=mybir.SyncInfo(on_wait=[w], on_update=[]),
                    bass_nofuse=True,
                )
                b.instructions.insert(pos, nop)

    nc.finalize()
    return nc


def pack_indices(center, context, neg_context, n_chunks=N_CHUNKS):
    """Pack per-core indices into the SBUF layouts the kernel expects.

    ctx_idx [P, n_chunks*CTX]: [p, c*CTX + k] = context[c*128 + p, k]
    w_idx   [P, n_chunks*11]:  [p, c*11 + 0] = center row, +1.. = negatives
    (w indices are offset by VOCAB into the concatenated table)
    """
    rows = n_chunks * P
    ctx_l, w_l = [], []
    for m in range(N_CORES):
        lo = m * rows
        ctx = np.ascontiguousarray(context[lo:lo + rows]).astype(np.int32)
        cen = np.ascontiguousarray(center[lo:lo + rows]).astype(np.int32)
        neg = np.ascontiguousarray(neg_context[lo:lo + rows]).astype(np.int32)
        ctx_p = ctx.reshape(n_chunks, P, CTX).transpose(1, 0, 2).reshape(P, n_chunks * CTX)
        w = np.concatenate([cen.reshape(rows, 1), neg.reshape(rows, K_NEG)], axis=1) + VOCAB
        w_p = w.reshape(n_chunks, P, W_COLS).transpose(1, 0, 2).reshape(P, n_chunks * W_COLS)
        ctx_l.append(np.ascontiguousarray(ctx_p))
        w_l.append(np.ascontiguousarray(w_p.astype(np.int32)))
    return ctx_l, w_l


def make_table(in_W, out_W):
    """fp8 e4m3 concat table; rows pre-scaled into e4m3's normal range
    (the raw values are denormal at fp8 and would lose most precision)."""
    import ml_dtypes

    tab = np.empty((2 * VOCAB, DIM), dtype=ml_dtypes.float8_e4m3)
    tab[:VOCAB] = (np.asarray(in_W, dtype=np.float32) * S_IN).astype(ml_dtypes.float8_e4m3)
    tab[VOCAB:] = (np.asarray(out_W, dtype=np.float32) * S_OUT).astype(ml_dtypes.float8_e4m3)
    return np.ascontiguousarray(tab)


def make_in_maps(center, context, neg_context, in_W, out_W):
    ctx_l, w_l = pack_indices(np.asarray(center), np.asarray(context), np.asarray(neg_context))
    tab = make_table(in_W, out_W)
    cf = SUPERS[0]
    in_maps = []
    for m in range(N_CORES):
        c, w = ctx_l[m], w_l[m]
        idx0 = np.ascontiguousarray(
            np.concatenate([c[:, :cf * CTX], w[:, :cf * W_COLS]], axis=1))
        idxr = np.ascontiguousarray(
            np.concatenate([c[:, cf * CTX:], w[:, cf * W_COLS:]], axis=1))
        in_maps.append({"idx0": idx0, "idxr": idxr, "tab": tab})
    return in_maps


def kernel(center, context, neg_context, in_W, out_W):
    from concourse.bass_utils import run_bass_kernel_spmd

    if "nc" not in _CACHE:
        _CACHE["nc"] = build_bass()
    nc = _CACHE["nc"]

    in_maps = make_in_maps(center, context, neg_context, in_W, out_W)
    # Rare per-core HW corruption (can be sticky on a given core) shows up
    # as NaN partials.  Retry with the slice->core assignment ROTATED each
    # attempt so a slice pinned to a bad core is recomputed by a good one.
    vals = np.full(N_CORES, np.nan)
    for attempt in range(2 * N_CORES):
        rot = attempt % N_CORES
        maps = [None] * N_CORES
        for s in range(N_CORES):
            maps[(s + rot) % N_CORES] = in_maps[s]
        res = run_bass_kernel_spmd(nc, maps, core_ids=list(range(N_CORES)))
        for s in range(N_CORES):
            if not np.isfinite(vals[s]):
                part = np.asarray(
                    res.results[(s + rot) % N_CORES]["loss"], dtype=np.float64
                )
                v = part[:, 0::2].sum() - part[:, 1::2].sum() / DOT_SCALE
                if np.isfinite(v):
                    vals[s] = v
        if np.isfinite(vals).all():
            break
    return np.float32(vals.sum() / BATCH)
